# revision 56
# baseline (speedup 1.0000x reference)
"""Multi-head attention (B=4, L=2048, D=1024, H=16, hd=64) on 8 NeuronCores.

Sharding: core = (batch b, head-group g) on a 4x2 grid. Each core handles one
batch and 8 heads (a 512-wide slice of the output dim). QKV projections are
column-parallel (each core only computes its own heads' projections), and
attention is fully local per (batch, head), so there are no collectives.

Host prep: shards are handed to each core with q/k/v pre-transposed to
c-major [D, L] bf16 and weights pre-transposed/pre-scaled, so every on-device
matmul operand is already contraction-major.

Per-core dataflow (all matmuls bf16 with fp32 PSUM accumulation):
  - Projections: KpT/QpT produced transposed [j, L] (bias is per-partition
    there); Vp produced natural [L, j] with a ones-column appended per head.
    Projection work is emitted in small chunks interleaved between attention
    iterations so it fills PE slack instead of starving ACT.
  - Attention per (head-pair, 512-wide q-chunk): each iteration computes one
    fresh [128, 1024] PSUM tile holding both heads' S' = Kp@Qp^T scores; the
    two matmuls sit on disjoint PE row groups (partitions 0-63 / 64-127) and
    run concurrently. One contiguous width-1024 exp on ACT writes E' (bf16),
    then O^T[65, 512] += Vp_aug^T-block @ E' (row 64 accumulates the softmax
    sums via the ones column).
  - Epilogue (deferred into the next group's slack): O^T leaves PSUM as bf16
    and the DMA xbar transposes it back to natural [q, *]; DVE computes
    1/sums and applies (O/sums + bv); fp32 result is stored. The V bias rides
    here because P@(V + 1*bv^T) normalized equals O/sums + bv.
"""

import numpy as np
import ml_dtypes

B, L, D = 4, 2048, 1024
H, HD = 16, 64
NCORES = 8
JG = 512  # output dims per core (8 heads * 64)
HPC = 8  # heads per core
CB = D // 128  # contraction blocks (8)
JB = JG // 128  # j blocks per core (4)
LB = L // 128  # l blocks (16)
KB = LB  # k blocks in attention (16)
SCALE = 1.0 / np.sqrt(HD)

_CACHE = {}

bf16 = ml_dtypes.bfloat16


def _build_program():
    from contextlib import ExitStack

    import concourse.bacc as bacc
    import concourse.bass as bass
    import concourse.tile as tile
    from concourse import mybir
    from concourse.masks import make_identity

    f32 = mybir.dt.float32
    bf = mybir.dt.bfloat16
    Exp = mybir.ActivationFunctionType.Exp
    add = mybir.AluOpType.add
    mult = mybir.AluOpType.mult

    nc = bacc.Bacc(
        "TRN2", target_bir_lowering=False, debug=False, enable_asserts=False
    )

    # q/k/v arrive pre-transposed to c-major [D, L] bf16
    q_in = nc.dram_tensor("q", [D, L], bf, kind="ExternalInput").ap()
    k_in = nc.dram_tensor("k", [D, L], bf, kind="ExternalInput").ap()
    v_in = nc.dram_tensor("v", [D, L], bf, kind="ExternalInput").ap()
    wqt_in = nc.dram_tensor("wqt", [D, JG], bf, kind="ExternalInput").ap()
    wkt_in = nc.dram_tensor("wkt", [D, JG], bf, kind="ExternalInput").ap()
    wvt_in = nc.dram_tensor("wvt", [D, JG], bf, kind="ExternalInput").ap()
    bq_in = nc.dram_tensor("bq", [JG], f32, kind="ExternalInput").ap()
    bk_in = nc.dram_tensor("bk", [JG], f32, kind="ExternalInput").ap()
    bv_in = nc.dram_tensor("bv", [JG], f32, kind="ExternalInput").ap()
    out = nc.dram_tensor("out", [L, JG], f32, kind="ExternalOutput").ap()

    with tile.TileContext(nc) as tc, ExitStack() as ctx:
        perm = ctx.enter_context(tc.tile_pool(name="perm", bufs=1))
        kqp = ctx.enter_context(tc.tile_pool(name="kqp", bufs=1))
        epool = ctx.enter_context(tc.tile_pool(name="epool", bufs=14))
        otp = ctx.enter_context(tc.tile_pool(name="otp", bufs=2))
        ostage = ctx.enter_context(tc.tile_pool(name="ostage", bufs=4))
        psS = ctx.enter_context(tc.tile_pool(name="psS", bufs=2, space="PSUM"))
        psO = ctx.enter_context(tc.tile_pool(name="psO", bufs=4, space="PSUM"))

        # ---- c-major activation tiles (plain loads; host pre-transposed) ----
        # kt rides the Sync HWDGE ring, qt the Scalar ring, and the V/weight
        # path rides SWDGE — three parallel DMA streams, no xbar involved.
        kt = []
        for cb in range(CB):
            t = kqp.tile([128, L], bf, name=f"kt{cb}", tag=f"kt{cb}")
            nc.sync.dma_start(out=t, in_=k_in[cb * 128 : (cb + 1) * 128, :])
            kt.append(t)
        qt = []
        for cb in range(CB):
            t = kqp.tile([128, L], bf, name=f"qt{cb}", tag=f"qt{cb}")
            nc.scalar.dma_start(out=t, in_=q_in[cb * 128 : (cb + 1) * 128, :])
            qt.append(t)

        # weights before vt: they gate the first projections
        wk_sb = perm.tile([128, CB, JG], bf, name="wk_sb", tag="wk")
        nc.gpsimd.dma_start(
            out=wk_sb, in_=wkt_in.rearrange("(cb p) j -> p cb j", p=128)
        )
        wq_sb = perm.tile([128, CB, JG], bf, name="wq_sb", tag="wq")
        nc.gpsimd.dma_start(
            out=wq_sb, in_=wqt_in.rearrange("(cb p) j -> p cb j", p=128)
        )
        bq_sb = perm.tile([128, JB], f32, name="bq_sb", tag="bq")
        nc.scalar.dma_start(out=bq_sb, in_=bq_in.rearrange("(jb p) -> p jb", p=128))
        bk_sb = perm.tile([128, JB], f32, name="bk_sb", tag="bk")
        nc.scalar.dma_start(out=bk_sb, in_=bk_in.rearrange("(jb p) -> p jb", p=128))

        # V path pool is released once all V-projection chunks are emitted.
        vp = [
            perm.tile([128, HPC, 65], bf, name=f"vp{lb}", tag=f"vp{lb}")
            for lb in range(LB)
        ]
        vtp = tc.alloc_tile_pool(name="vtp", bufs=1)
        wv_sb = vtp.tile([128, CB, JG], bf, name="wv_sb", tag="wv")
        nc.gpsimd.dma_start(
            out=wv_sb, in_=wvt_in.rearrange("(cb p) j -> p cb j", p=128)
        )
        vt = []
        vt_eng = [nc.sync, nc.sync, nc.sync, nc.scalar, nc.scalar, nc.scalar,
                  nc.gpsimd, nc.gpsimd]
        for cb in range(CB):
            t = vtp.tile([128, L], bf, name=f"vt{cb}", tag=f"vt{cb}")
            vt_eng[cb].dma_start(out=t, in_=v_in[cb * 128 : (cb + 1) * 128, :])
            vt.append(t)

        # ---- persistent small tensors ----
        # Per-pair V-bias broadcast tiles [128, 128] f32; the V bias is applied
        # AFTER the softmax division (P@(V + 1*bv^T) row-normalized equals
        # O_unnorm/sums + bv), which keeps it off the PV critical path.
        bvb = []
        for jb in range(JB):
            t = perm.tile([128, 128], f32, name=f"bvb{jb}", tag=f"bvb{jb}")
            seg = bv_in[jb * 128 : (jb + 1) * 128]
            nc.gpsimd.dma_start(
                out=t,
                in_=bass.AP(
                    tensor=seg.tensor, offset=seg.offset, ap=[[0, 128]] + list(seg.ap)
                ),
            )
            bvb.append(t)
        # preload the exp table set during the DMA phase
        warm = perm.tile([128, 1], f32, name="warm", tag="warm")
        nc.vector.memset(warm, 0.0)
        nc.scalar.activation(warm, warm, Exp)

        kpt = [
            perm.tile([128, L], bf, name=f"kpt{jb}", tag=f"kpt{jb}")
            for jb in range(JB)
        ]
        qpt = [
            perm.tile([128, L], bf, name=f"qpt{jb}", tag=f"qpt{jb}")
            for jb in range(JB)
        ]

        # ---- projection work, emitted in small chunks so the scheduler can
        # slot it into PE slack between attention iterations ----
        def v_chunk(lb):
            def emit():
                # V projection: natural layout [l, j], ones column per head;
                # bv is folded in after the softmax division instead.
                vps = psO.tile([128, JG], f32, name="vps", tag="O")
                for cb in range(CB):
                    nc.tensor.matmul(
                        vps,
                        lhsT=vt[cb][:, lb * 128 : (lb + 1) * 128],
                        rhs=wv_sb[:, cb],
                        start=(cb == 0),
                        stop=(cb == CB - 1),
                    )
                nc.vector.tensor_copy(
                    out=vp[lb][:, :, 0:64],
                    in_=vps.rearrange("p (h d) -> p h d", h=HPC),
                )
                nc.gpsimd.memset(vp[lb][:, :, 64:65], 1.0)

            return emit

        def kq_chunk(which, jb, lc):
            """One projection output chunk, split into two emitters (4+4
            contraction blocks) so interleaved projection work never bursts
            long enough on PE to starve ACT."""
            state = {}

            def sel():
                return (
                    (wk_sb, kt, bk_sb, kpt[jb])
                    if which == "k"
                    else (wq_sb, qt, bq_sb, qpt[jb])
                )

            def part(c_lo, c_hi, final):
                def emit():
                    w_sb, x_t, b_sb, dst = sel()
                    if c_lo == 0:
                        state["ps"] = psO.tile(
                            [128, 512], f32, name="kqps", tag="O"
                        )
                    ps = state["ps"]
                    for cb in range(c_lo, c_hi):
                        nc.tensor.matmul(
                            ps,
                            lhsT=w_sb[:, cb, jb * 128 : (jb + 1) * 128],
                            rhs=x_t[cb][:, lc * 512 : (lc + 1) * 512],
                            start=(cb == 0),
                            stop=(cb == CB - 1),
                        )
                    if final:
                        nc.vector.tensor_scalar_add(
                            dst[:, lc * 512 : (lc + 1) * 512],
                            ps,
                            b_sb[:, jb : jb + 1],
                        )

                return emit

            return [part(0, 4, False), part(4, CB, True)]

        def attn(jb, qh, extras=None, pv_lag=5):
            q0 = qh * 512
            qs = slice(q0, q0 + 512)
            oacc = [
                psO.tile([65, 512], f32, name=f"oacc{hh}", tag="O")
                for hh in range(2)
            ]
            def pv(kb, hh):
                nc.tensor.matmul(
                    oacc[hh],
                    lhsT=vp[kb][:, 2 * jb + hh, :],
                    rhs=es[kb][:, hh * 512 : (hh + 1) * 512],
                    start=(kb == 0),
                    stop=(kb == KB - 1),
                )

            es = {}
            for kb in range(KB):
                for emit in (extras or {}).get(kb, ()):
                    emit()
                ks = slice(kb * 128, (kb + 1) * 128)
                # One fresh [128, 1024] tile per iteration holds both heads'
                # scores (head A cols 0-511, head B cols 512-1023). The two
                # matmuls sit on disjoint PE row groups and different PSUM
                # banks, so they run concurrently; one contiguous width-1024
                # exp covers both.
                s = psS.tile([128, 1024], f32, name="s", tag="s")
                for hh in range(2):
                    hp = slice(hh * 64, (hh + 1) * 64)
                    nc.tensor.matmul(
                        s[:, hh * 512 : (hh + 1) * 512],
                        lhsT=kpt[jb][hp, ks],
                        rhs=qpt[jb][hp, qs],
                        start=True,
                        stop=True,
                    )
                e = epool.tile([128, 1024], bf, name="e", tag="e")
                nc.scalar.activation(e, s, Exp)
                es[kb] = e
                # PV matmuls lag by pv_lag iterations (first group: gives the
                # V projection time to land); the trailing ones are deferred
                # past the group boundary so they don't sit between this
                # group's last exp and the next group's S'
                x = kb - pv_lag
                if 0 <= x <= KB - 2:
                    for hh in range(2):
                        pv(x, hh)

            def tail_pv_piece(xs):
                def emit():
                    for x in xs:
                        for hh in range(2):
                            pv(x, hh)

                return emit

            rest = list(range(min(KB - 2, KB - 1 - pv_lag) + 1, KB))
            tail_pvs = [tail_pv_piece(rest[i : i + 2]) for i in range(0, len(rest), 2)]

            def tail_out(hh):
                # O^T leaves PSUM as bf16 (frees the accumulator) and the DMA
                # xbar transposes it to natural layout ([80, 512] ->
                # [128, 4, 80]; rows 65-79 pad).
                def emit():
                    ot = otp.tile([80, 512], bf, name=f"ot{hh}", tag="ot")
                    nc.vector.tensor_copy(out=ot[0:65, :], in_=oacc[hh])
                    onat = otp.tile([128, 4, 80], bf, name=f"onat{hh}", tag="onat")
                    nc.sync.dma_start(out=onat, in_=ot, transpose=True)
                    onats.append(onat)

                return emit

            onats = []

            def epi_piece(i):
                def emit():
                    og = ostage.tile([128, 128], f32, name="og", tag="og")
                    for hh in range(2):
                        rec = ostage.tile([128, 1], f32, name="rec", tag="rec")
                        nc.vector.reciprocal(rec, onats[hh][:, i, 64:65])
                        # og = O_unnorm / sums + bv
                        nc.vector.scalar_tensor_tensor(
                            out=og[:, hh * 64 : (hh + 1) * 64],
                            in0=onats[hh][:, i, 0:64],
                            scalar=rec,
                            in1=bvb[jb][:, hh * 64 : (hh + 1) * 64],
                            op0=mult,
                            op1=add,
                        )
                    nc.sync.dma_start(
                        out=out[
                            q0 + i * 128 : q0 + (i + 1) * 128,
                            jb * 128 : (jb + 1) * 128,
                        ],
                        in_=og,
                    )

                return emit

            return tail_pvs + [tail_out(0), tail_out(1)] + [
                epi_piece(i) for i in range(4)
            ]

        # ---- emission schedule ----
        # attn(0, 0) carries the V projection and the rest of K0/Q0 as
        # interleaved chunks; later groups carry the previous group's output
        # epilogue plus upcoming projection chunks, one piece per iteration,
        # so PE work stays evenly spread and ACT never starves.
        for em in kq_chunk("k", 0, 0) + kq_chunk("q", 0, 0):
            em()
        # First group runs with pv_lag=3 so its first iterations are pure
        # S'/exp (no V dependency); V chunks sit one iteration ahead of their
        # (lagged) PV consumers, aligned with the vt DMA arrival.
        sched0 = {}
        for lb in range(14):
            sched0.setdefault(lb + 2, []).append(v_chunk(lb))
        sched0[15].extend([v_chunk(14), v_chunk(15)])
        for kb0, ch in ((0, ("k", 0, 1)), (5, ("k", 0, 2)), (9, ("k", 0, 3)), (12, ("q", 0, 1))):
            for off, em in enumerate(kq_chunk(*ch)):
                sched0.setdefault(kb0 + off, []).append(em)
        carry = attn(0, 0, sched0, pv_lag=3)
        vtp.release()

        def run(jb, qh, proj_parts):
            nonlocal carry
            items = list(carry) + list(proj_parts)
            extras = {}
            for idx, em in enumerate(items):
                extras.setdefault(min(idx + 1, 15), []).append(em)
            carry = attn(jb, qh, extras)

        for jb in range(JB):
            if jb > 0:
                run(jb, 0, kq_chunk("q", jb, 1))
            run(jb, 1, kq_chunk("q", jb, 2) + kq_chunk("q", jb, 3))
            if jb + 1 < JB:
                run(
                    jb,
                    2,
                    kq_chunk("k", jb + 1, 0)
                    + kq_chunk("k", jb + 1, 1)
                    + kq_chunk("k", jb + 1, 2)
                    + kq_chunk("k", jb + 1, 3),
                )
                run(jb, 3, kq_chunk("q", jb + 1, 0))
            else:
                run(jb, 2, [])
                run(jb, 3, [])
        for em in carry:
            em()

    nc.compile()
    return nc


def _prep_inputs(q, k, v, Wq, bq, Wk, bk, Wv, bv):
    """Shard across the 4x2 (batch, head-group) grid. Activations are cast to
    bf16 and transposed to c-major; weights pre-transposed to c-major and
    pre-scaled by 1/sqrt(hd) on the Q side."""
    as_np = lambda a: np.asarray(a, dtype=np.float32)
    q, k, v = as_np(q), as_np(k), as_np(v)
    Wq, bq, Wk, bk, Wv, bv = map(as_np, (Wq, bq, Wk, bk, Wv, bv))

    qT = [np.ascontiguousarray(q[b].T).astype(bf16) for b in range(B)]
    kT = [np.ascontiguousarray(k[b].T).astype(bf16) for b in range(B)]
    vT = [np.ascontiguousarray(v[b].T).astype(bf16) for b in range(B)]

    in_maps = []
    for core in range(NCORES):
        b, g = divmod(core, 2)
        js = slice(g * JG, (g + 1) * JG)
        in_maps.append(
            {
                "q": qT[b],
                "k": kT[b],
                "v": vT[b],
                "wqt": np.ascontiguousarray((Wq[js] * SCALE).T).astype(bf16),
                "wkt": np.ascontiguousarray(Wk[js].T).astype(bf16),
                "wvt": np.ascontiguousarray(Wv[js].T).astype(bf16),
                "bq": np.ascontiguousarray(bq[js] * SCALE),
                "bk": np.ascontiguousarray(bk[js]),
                "bv": np.ascontiguousarray(bv[js]),
            }
        )
    return in_maps


def kernel(q, k, v, Wq, bq, Wk, bk, Wv, bv, trace=False):
    from concourse.bass_utils import run_bass_kernel_spmd

    if "nc" not in _CACHE:
        _CACHE["nc"] = _build_program()
    nc = _CACHE["nc"]

    in_maps = _prep_inputs(q, k, v, Wq, bq, Wk, bk, Wv, bv)
    res = run_bass_kernel_spmd(
        nc, in_maps, core_ids=list(range(NCORES)), trace=trace
    )
    _CACHE["last_results"] = res

    full = np.empty((B, L, D), dtype=np.float32)
    for core in range(NCORES):
        b, g = divmod(core, 2)
        full[b, :, g * JG : (g + 1) * JG] = res.results[core]["out"]
    return full
